# revision 10
# baseline (speedup 1.0000x reference)
"""Trainium2 Bass kernel for nn_Grid1 (embedding_lookup / grid resample).

Math: the reference is torch-style grid_sample(bilinear, border, align_corners=True)
on a coordinate lattice that is an integer pixel lattice wrapped mod 1024:

    out[0, c, i, j] = grid[0, c, (ys + i) % 1024, (xs + j) % 1024]

(the normalized-coordinate round trip maps every sample to within 6.1e-5 of an
exact integer pixel, so bilinear weights degenerate to a pure gather; measured
L2 rel err of the pure gather vs the f32 reference is ~4e-5).

The 4096x4096 output is therefore a 4x4 periodic tiling of the (ys, xs)-rolled
1024x1024 grid. Sharding: each of the 8 cores owns one 128-row class of the
rolled grid (rows [128k, 128(k+1)) of the period), reads only its 4ch x 128 x 1024
band (2MB), and writes its 16 output blocks (4 vertical periods x 4 horizontal
periods worth, 32MB). The x-roll happens on-device via two segment DMAs per
block; HBM traffic per core = 2MB read + 32MB write (~memory roofline).
"""

from contextlib import ExitStack

import numpy as np

from concourse import bass, mybir
from concourse.bass_utils import run_bass_kernel_spmd

C = 4          # channels
G = 1024       # grid height/width (period)
HOUT = 4096    # output height/width
NCORES = 8
PB = G // NCORES      # rows of the period per core = 128 (= SBUF partitions)
V = HOUT // G         # vertical period repeats = 4
R = HOUT // G         # horizontal period repeats = 4

_NC_CACHE: dict = {}

# Set by test harnesses to capture an NTFF profile; harmless default.
TRACE = False
LAST_RESULT = None


# 1 = proven (2-segment rolled stores); 3 = experimental (replicated-row
# contiguous stores); 4 = host x-roll + DVE 4x replication + one 8MB
# store per channel (16KB contiguous descriptors), loads on the scalar
# HWDGE ring so they don't serialize ahead of stores; 5 = v4 + dual-ring
# stores + 3-engine parallel replication; 6 = v5 minus gpsimd (its Q7
# copy takes 4.1us and gated the first store; DVE does 2 slots instead);
# 7 = v6 + anti-straggler store split (SDMA engine 15 runs ~19% slow on
# some cores/runs; deal it ~19% fewer bytes via a 3-way store split that
# exploits the blocked ceil(npart/16)-partitions-per-engine dealing).
KERNEL_VERSION = 7

# Column split for v7: engine 15 (the occasionally-slow SDMA engine)
# only serves partitions 120-127 of the full-width store A, i.e. WA of
# 4096 columns; partitions 0-119 finish their tail in store B (15
# engines), partitions 120-127 finish theirs in store C (engines 0-7).
WA = 3328


def _build(xs: int) -> bass.Bass:
    if KERNEL_VERSION == 7:
        return _build_v7()
    if KERNEL_VERSION == 6:
        return _build_v6()
    if KERNEL_VERSION == 5:
        return _build_v5()
    if KERNEL_VERSION == 4:
        return _build_v4()
    if KERNEL_VERSION == 3:
        return _build_v3(xs)
    return _build_v1(xs)


def _build_v7() -> bass.Bass:
    """v6 + anti-straggler split. The HWDGE deals a DMA's partitions to
    SDMA engines in blocks of ceil(npart/16), starting at engine 0
    (probe-verified). SDMA engine 15 runs ~0.81x speed on some
    cores/runs, so per channel the store is split:

      A: p[0:128)  cols [0:WA)    -> engine j <- partitions [8j, 8j+8);
                                     engine 15 sees only 8 x WA
      B: p[0:120)  cols [WA:4096) -> engines 0-14 (8 partitions each)
      C: p[120:128) cols [WA:4096) -> engines 0-7 (1 partition each)

    Engine 15 carries WA/4096 = 81% of uniform; engines 0-7 carry
    ~102.3%. Healthy-run cost ~+2%, slow-run saving ~-16%.
    """
    EXT = HOUT
    nc = bass.Bass()
    g = nc.declare_dram_parameter("g", [C, PB, G], mybir.dt.float32, isOutput=False)
    o = nc.declare_dram_parameter("o", [C, V, PB, HOUT], mybir.dt.float32,
                                  isOutput=True)
    with ExitStack() as ctx:
        block = ctx.enter_context(nc.Block(no_gpsimd_drain=True))
        ld_sems = [ctx.enter_context(nc.semaphore(f"ld{c}")) for c in range(C)]
        full_sems = [ctx.enter_context(nc.semaphore(f"full{c}")) for c in range(C)]
        st_sem = ctx.enter_context(nc.semaphore("st"))
        t = ctx.enter_context(nc.sbuf_tensor("t", [PB, C * EXT], mybir.dt.float32))

        def slot(c, k):
            return t[:, c * EXT + k * G:c * EXT + (k + 1) * G]

        def store(eng, c):
            eng.wait_ge(full_sems[c], 3)
            base = c * EXT
            # A: all partitions, cols [0:WA)
            dst = o[c][:, :, 0:WA].rearrange("v p col -> p v col")
            src = t[:, base:base + WA].unsqueeze(1).broadcast_to((PB, V, WA))
            eng.dma_start(dst, src).then_inc(st_sem, 16)
            # B: partitions [0:120), cols [WA:4096)
            dst = o[c][:, 0:120, WA:HOUT].rearrange("v p col -> p v col")
            src = t[0:120, base + WA:base + EXT]
            src = src.unsqueeze(1).broadcast_to((120, V, EXT - WA))
            eng.dma_start(dst, src).then_inc(st_sem, 16)
            # C: partitions [120:128), cols [WA:4096)
            dst = o[c][:, 120:PB, WA:HOUT].rearrange("v p col -> p v col")
            src = t[120:PB, base + WA:base + EXT]
            src = src.unsqueeze(1).broadcast_to((PB - 120, V, EXT - WA))
            eng.dma_start(dst, src).then_inc(st_sem, 16)

        @block.vector
        def _(vector: bass.BassEngine):
            for c in range(C):
                vector.wait_ge(ld_sems[c], 16)
                vector.tensor_copy(slot(c, 1), slot(c, 0)).then_inc(
                    full_sems[c], 1)
                vector.tensor_copy(slot(c, 3), slot(c, 0)).then_inc(
                    full_sems[c], 1)

        @block.scalar
        def _(scalar: bass.BassEngine):
            for c in range(C):
                scalar.dma_start(slot(c, 0), g[c]).then_inc(ld_sems[c], 16)
            for c in range(C):
                scalar.wait_ge(ld_sems[c], 16)
                scalar.copy(slot(c, 2), slot(c, 0)).then_inc(full_sems[c], 1)
                if c in (1, 3):
                    store(scalar, c)

        @block.sync
        def _(sync: bass.BassEngine):
            store(sync, 0)
            store(sync, 2)
            sync.wait_ge(st_sem, 16 * 3 * C)
    return nc


def _build_v6() -> bass.Bass:
    """v5 with the replication split DVE(2 slots) + scalar(1 slot); the
    gpsimd Q7 copy (4.1us for 512KB) was gating the first store."""
    EXT = HOUT
    nc = bass.Bass()
    g = nc.declare_dram_parameter("g", [C, PB, G], mybir.dt.float32, isOutput=False)
    o = nc.declare_dram_parameter("o", [C, V, PB, HOUT], mybir.dt.float32,
                                  isOutput=True)
    with ExitStack() as ctx:
        block = ctx.enter_context(nc.Block(no_gpsimd_drain=True))
        ld_sems = [ctx.enter_context(nc.semaphore(f"ld{c}")) for c in range(C)]
        full_sems = [ctx.enter_context(nc.semaphore(f"full{c}")) for c in range(C)]
        st_sem = ctx.enter_context(nc.semaphore("st"))
        t = ctx.enter_context(nc.sbuf_tensor("t", [PB, C * EXT], mybir.dt.float32))

        def slot(c, k):
            return t[:, c * EXT + k * G:c * EXT + (k + 1) * G]

        def store(eng, c):
            eng.wait_ge(full_sems[c], 3)
            dst = o[c].rearrange("v p col -> p v col")
            src = t[:, c * EXT:(c + 1) * EXT]
            src = src.unsqueeze(1).broadcast_to((PB, V, EXT))
            eng.dma_start(dst, src).then_inc(st_sem, 16)

        @block.vector
        def _(vector: bass.BassEngine):
            for c in range(C):
                vector.wait_ge(ld_sems[c], 16)
                vector.tensor_copy(slot(c, 1), slot(c, 0)).then_inc(
                    full_sems[c], 1)
                vector.tensor_copy(slot(c, 3), slot(c, 0)).then_inc(
                    full_sems[c], 1)

        @block.scalar
        def _(scalar: bass.BassEngine):
            for c in range(C):
                scalar.dma_start(slot(c, 0), g[c]).then_inc(ld_sems[c], 16)
            for c in range(C):
                scalar.wait_ge(ld_sems[c], 16)
                scalar.copy(slot(c, 2), slot(c, 0)).then_inc(full_sems[c], 1)
                if c in (1, 3):
                    store(scalar, c)

        @block.sync
        def _(sync: bass.BassEngine):
            store(sync, 0)
            store(sync, 2)
            sync.wait_ge(st_sem, 16 * C)
    return nc


def _build_v5() -> bass.Bass:
    """v4 + (a) stores split across both HWDGE rings (sync: c0,c2;
    scalar: c1,c3) so descriptor fetch/decode overlaps across rings,
    and (b) the 3 replication copies per channel run on DVE, scalar and
    gpsimd in parallel, shortening the ld0 -> replicate -> store chain.
    """
    EXT = HOUT
    nc = bass.Bass()
    g = nc.declare_dram_parameter("g", [C, PB, G], mybir.dt.float32, isOutput=False)
    o = nc.declare_dram_parameter("o", [C, V, PB, HOUT], mybir.dt.float32,
                                  isOutput=True)
    with ExitStack() as ctx:
        block = ctx.enter_context(nc.Block())
        ld_sems = [ctx.enter_context(nc.semaphore(f"ld{c}")) for c in range(C)]
        full_sems = [ctx.enter_context(nc.semaphore(f"full{c}")) for c in range(C)]
        st_sem = ctx.enter_context(nc.semaphore("st"))
        t = ctx.enter_context(nc.sbuf_tensor("t", [PB, C * EXT], mybir.dt.float32))

        def slot(c, k):
            return t[:, c * EXT + k * G:c * EXT + (k + 1) * G]

        def store(eng, c):
            eng.wait_ge(full_sems[c], 3)
            dst = o[c].rearrange("v p col -> p v col")
            src = t[:, c * EXT:(c + 1) * EXT]
            src = src.unsqueeze(1).broadcast_to((PB, V, EXT))
            eng.dma_start(dst, src).then_inc(st_sem, 16)

        @block.vector
        def _(vector: bass.BassEngine):
            for c in range(C):
                vector.wait_ge(ld_sems[c], 16)
                vector.tensor_copy(slot(c, 1), slot(c, 0)).then_inc(
                    full_sems[c], 1)

        @block.gpsimd
        def _(gpsimd: bass.BassEngine):
            for c in range(C):
                gpsimd.wait_ge(ld_sems[c], 16)
                gpsimd.tensor_copy(slot(c, 3), slot(c, 0)).then_inc(
                    full_sems[c], 1)

        @block.scalar
        def _(scalar: bass.BassEngine):
            for c in range(C):
                scalar.dma_start(slot(c, 0), g[c]).then_inc(ld_sems[c], 16)
            for c in range(C):
                scalar.wait_ge(ld_sems[c], 16)
                scalar.copy(slot(c, 2), slot(c, 0)).then_inc(full_sems[c], 1)
                if c in (1, 3):
                    store(scalar, c)

        @block.sync
        def _(sync: bass.BassEngine):
            store(sync, 0)
            store(sync, 2)
            sync.wait_ge(st_sem, 16 * C)
    return nc


def _build_v4() -> bass.Bass:
    """Host pre-rolls columns, so SBUF rows are already output-ordered.

    Per core: load the (C, PB, G) band (2MB) via the scalar HWDGE ring,
    DVE-replicates each channel row 4x side-by-side (4KB -> 16KB), then
    one store DMA per channel writes the (V, PB, 16KB) block with the
    v-replication done by a stride-0 source dim. Descriptors are 16KB
    contiguous -> minimal per-packet overhead; the HBM write cap
    (~358 GB/s/core) becomes the binding limit.
    """
    EXT = HOUT  # 4096 replicated columns per channel
    nc = bass.Bass()
    g = nc.declare_dram_parameter("g", [C, PB, G], mybir.dt.float32, isOutput=False)
    o = nc.declare_dram_parameter("o", [C, V, PB, HOUT], mybir.dt.float32,
                                  isOutput=True)
    with ExitStack() as ctx:
        block = ctx.enter_context(nc.Block())
        ld_sems = [ctx.enter_context(nc.semaphore(f"ld{c}")) for c in range(C)]
        dve_sem = ctx.enter_context(nc.semaphore("dve"))
        st_sem = ctx.enter_context(nc.semaphore("st"))
        t = ctx.enter_context(nc.sbuf_tensor("t", [PB, C * EXT], mybir.dt.float32))

        @block.scalar
        def _(scalar: bass.BassEngine):
            # Loads ride the Act HWDGE ring; stores ride the SP ring, so
            # the 4 loads never queue behind 8MB store descriptors.
            for c in range(C):
                scalar.dma_start(t[:, c * EXT:c * EXT + G], g[c]).then_inc(
                    ld_sems[c], 16)

        @block.vector
        def _(vector: bass.BassEngine):
            for c in range(C):
                base = c * EXT
                vector.wait_ge(ld_sems[c], 16)
                inst = None
                for k in range(1, EXT // G):
                    inst = vector.tensor_copy(
                        t[:, base + k * G:base + (k + 1) * G],
                        t[:, base:base + G],
                    )
                inst.then_inc(dve_sem, 1)

        @block.sync
        def _(sync: bass.BassEngine):
            for c in range(C):
                sync.wait_ge(dve_sem, c + 1)
                # dst (p, v, col): one 8MB DMA, 16KB contiguous per (p, v).
                dst = o[c].rearrange("v p col -> p v col")
                src = t[:, c * EXT:(c + 1) * EXT]
                src = src.unsqueeze(1).broadcast_to((PB, V, EXT))
                sync.dma_start(dst, src).then_inc(st_sem, 16)
            sync.wait_ge(st_sem, 16 * C)
    return nc


def _build_v3(xs: int) -> bass.Bass:
    """One SPMD program, specialized on the column shift xs.

    Raw bass (not Tile): the static-DMA lowering in this toolchain only
    supports a single sync-wait per DMA instruction, so sequencer-side
    wait_ge + per-channel load semaphores are used instead of Tile's
    auto-generated multi-sem waits.

    v3: each channel's grid row is replicated 5x side-by-side in SBUF
    (DVE copies — otherwise idle), so every output row is one contiguous
    16KB descriptor ext[p, xs:xs+4096] and each (c, v) block is a single
    2MB store DMA with maximal descriptor size. (v1 used 2KB segmented
    descriptors from the column roll; engines ran at 23.3/27 GB/s and
    the slow 16th engine set a 117us makespan.)
    """
    EXT = G + HOUT  # 5120 replicated columns per channel
    nc = bass.Bass()
    g = nc.declare_dram_parameter("g", [C, PB, G], mybir.dt.float32, isOutput=False)
    o = nc.declare_dram_parameter("o", [C, V, PB, HOUT], mybir.dt.float32, isOutput=True)
    with ExitStack() as ctx:
        block = ctx.enter_context(nc.Block())
        ld_sems = [ctx.enter_context(nc.semaphore(f"ld{c}")) for c in range(C)]
        dve_sem = ctx.enter_context(nc.semaphore("dve"))
        st_sem = ctx.enter_context(nc.semaphore("st"))
        t = ctx.enter_context(nc.sbuf_tensor("t", [PB, C * EXT], mybir.dt.float32))

        @block.vector
        def _(vector: bass.BassEngine):
            for c in range(C):
                base = c * EXT
                vector.wait_ge(ld_sems[c], 16)
                for k in range(1, EXT // G):
                    inst = vector.tensor_copy(
                        t[:, base + k * G:base + (k + 1) * G],
                        t[:, base:base + G],
                    )
                inst.then_inc(dve_sem, 1)

        @block.sync
        def _(sync: bass.BassEngine):
            for c in range(C):
                sync.dma_start(t[:, c * EXT:c * EXT + G], g[c]).then_inc(
                    ld_sems[c], 16)
            for c in range(C):
                sync.wait_ge(dve_sem, c + 1)
                for v in range(V):
                    src = t[:, c * EXT + xs:c * EXT + xs + HOUT]
                    sync.dma_start(o[c, v], src).then_inc(st_sem, 16)
            sync.wait_ge(st_sem, 16 * C * V)
    return nc


def _build_v1(xs: int) -> bass.Bass:
    """v1 (kept for reference): column roll via 2-segment stores."""
    nc = bass.Bass()
    g = nc.declare_dram_parameter("g", [C, PB, G], mybir.dt.float32, isOutput=False)
    o = nc.declare_dram_parameter("o", [C, V, PB, HOUT], mybir.dt.float32, isOutput=True)
    L = G - xs
    with ExitStack() as ctx:
        block = ctx.enter_context(nc.Block())
        ld_sems = [ctx.enter_context(nc.semaphore(f"ld{c}")) for c in range(C)]
        st_sem = ctx.enter_context(nc.semaphore("st"))
        t = ctx.enter_context(nc.sbuf_tensor("t", [PB, C * G], mybir.dt.float32))

        @block.sync
        def _(sync: bass.BassEngine):
            for c in range(C):
                sync.dma_start(t[:, c * G:(c + 1) * G], g[c]).then_inc(
                    ld_sems[c], 16)
            nstores = 0
            for c in range(C):
                sync.wait_ge(ld_sems[c], 16)
                for v in range(V):
                    # out[c, v, p, r*1024 + b] = t[p, c*1024 + (xs + b) % 1024]
                    dst = o[c, v].rearrange("p (r col) -> p r col", col=G)
                    srcA = t[:, c * G + xs:(c + 1) * G]
                    srcA = srcA.unsqueeze(1).broadcast_to((PB, R, L))
                    sync.dma_start(dst[:, :, 0:L], srcA).then_inc(st_sem, 16)
                    nstores += 1
                    if xs:
                        srcB = t[:, c * G:c * G + xs]
                        srcB = srcB.unsqueeze(1).broadcast_to((PB, R, xs))
                        sync.dma_start(dst[:, :, L:G], srcB).then_inc(st_sem, 16)
                        nstores += 1
            sync.wait_ge(st_sem, 16 * nstores)
    return nc


def _get_nc(xs: int) -> bass.Bass:
    if KERNEL_VERSION >= 4:
        xs = 0  # v4+ rolls columns on the host; the NEFF is xs-independent
    key = (KERNEL_VERSION, xs)
    if key not in _NC_CACHE:
        _NC_CACHE[key] = _build(xs)
    return _NC_CACHE[key]


def kernel(grid, coordinate_start, h, w, support_resolution_h, support_resolution_w,
           **_unused):
    grid = np.asarray(grid, dtype=np.float32)
    cs = np.asarray(coordinate_start).astype(np.int64)
    xs = int(cs[0]) % G
    ys = int(cs[1]) % G
    assert grid.shape == (1, C, G, G), grid.shape
    assert int(h) == HOUT and int(w) == HOUT
    assert int(support_resolution_h) == G and int(support_resolution_w) == G

    g0 = grid[0]  # (C, G, G)
    if KERNEL_VERSION >= 4:
        # v4 does no on-device column roll: pre-roll the whole grid once
        # so band[c, p, j] = g0[c, rows[p], (xs + j) % G].
        g0 = np.ascontiguousarray(np.roll(g0, -xs, axis=2))
    in_maps = []
    for k in range(NCORES):
        rows = (ys + PB * k + np.arange(PB)) % G
        band = np.ascontiguousarray(g0[:, rows, :])  # (C, PB, G)
        in_maps.append({"g": band})

    nc = _get_nc(xs)
    res = run_bass_kernel_spmd(nc, in_maps, core_ids=list(range(NCORES)),
                               trace=TRACE)
    global LAST_RESULT
    LAST_RESULT = res

    full = np.empty((1, C, HOUT, HOUT), dtype=np.float32)
    for k in range(NCORES):
        r = np.asarray(res.results[k]["o"])  # (C, V, PB, HOUT)
        for v in range(V):
            base = v * G + PB * k
            full[0, :, base:base + PB, :] = r[:, v]
    return full



# revision 11
# speedup vs baseline: 1.0451x; 1.0451x over previous
"""Trainium2 Bass kernel for nn_Grid1 (embedding_lookup / grid resample).

Math: the reference is torch-style grid_sample(bilinear, border, align_corners=True)
on a coordinate lattice that is an integer pixel lattice wrapped mod 1024:

    out[0, c, i, j] = grid[0, c, (ys + i) % 1024, (xs + j) % 1024]

(the normalized-coordinate round trip maps every sample to within 6.1e-5 of an
exact integer pixel, so bilinear weights degenerate to a pure gather; measured
L2 rel err of the pure gather vs the f32 reference is ~4e-5).

The 4096x4096 output is therefore a 4x4 periodic tiling of the (ys, xs)-rolled
1024x1024 grid. Sharding: each of the 8 cores owns one 128-row class of the
rolled grid (rows [128k, 128(k+1)) of the period), reads only its 4ch x 128 x 1024
band (2MB), and writes its 16 output blocks (4 vertical periods x 4 horizontal
periods worth, 32MB). The x-roll happens on-device via two segment DMAs per
block; HBM traffic per core = 2MB read + 32MB write (~memory roofline).
"""

from contextlib import ExitStack

import numpy as np

from concourse import bass, mybir
from concourse.bass_utils import run_bass_kernel_spmd

C = 4          # channels
G = 1024       # grid height/width (period)
HOUT = 4096    # output height/width
NCORES = 8
PB = G // NCORES      # rows of the period per core = 128 (= SBUF partitions)
V = HOUT // G         # vertical period repeats = 4
R = HOUT // G         # horizontal period repeats = 4

_NC_CACHE: dict = {}

# Set by test harnesses to capture an NTFF profile; harmless default.
TRACE = False
LAST_RESULT = None


# 1 = proven (2-segment rolled stores); 3 = experimental (replicated-row
# contiguous stores); 4 = host x-roll + DVE 4x replication + one 8MB
# store per channel (16KB contiguous descriptors), loads on the scalar
# HWDGE ring so they don't serialize ahead of stores; 5 = v4 + dual-ring
# stores + 3-engine parallel replication; 6 = v5 minus gpsimd (its Q7
# copy takes 4.1us and gated the first store; DVE does 2 slots instead);
# 7 = v6 + anti-straggler store split (SDMA engine 15 runs ~19% slow on
# some cores/runs; deal it ~19% fewer bytes via a 3-way store split that
# exploits the blocked ceil(npart/16)-partitions-per-engine dealing).
# v7 REJECTED on HW: 13312B descriptors are ~8% less efficient than
# 16384B and the straggler engine is random per run (seen on 66/70/74
# too), so the static split loses both ways.
KERNEL_VERSION = 6

# Column split for v7: engine 15 (the occasionally-slow SDMA engine)
# only serves partitions 120-127 of the full-width store A, i.e. WA of
# 4096 columns; partitions 0-119 finish their tail in store B (15
# engines), partitions 120-127 finish theirs in store C (engines 0-7).
WA = 3328


def _build(xs: int) -> bass.Bass:
    if KERNEL_VERSION == 7:
        return _build_v7()
    if KERNEL_VERSION == 6:
        return _build_v6()
    if KERNEL_VERSION == 5:
        return _build_v5()
    if KERNEL_VERSION == 4:
        return _build_v4()
    if KERNEL_VERSION == 3:
        return _build_v3(xs)
    return _build_v1(xs)


def _build_v7() -> bass.Bass:
    """v6 + anti-straggler split. The HWDGE deals a DMA's partitions to
    SDMA engines in blocks of ceil(npart/16), starting at engine 0
    (probe-verified). SDMA engine 15 runs ~0.81x speed on some
    cores/runs, so per channel the store is split:

      A: p[0:128)  cols [0:WA)    -> engine j <- partitions [8j, 8j+8);
                                     engine 15 sees only 8 x WA
      B: p[0:120)  cols [WA:4096) -> engines 0-14 (8 partitions each)
      C: p[120:128) cols [WA:4096) -> engines 0-7 (1 partition each)

    Engine 15 carries WA/4096 = 81% of uniform; engines 0-7 carry
    ~102.3%. Healthy-run cost ~+2%, slow-run saving ~-16%.
    """
    EXT = HOUT
    nc = bass.Bass()
    g = nc.declare_dram_parameter("g", [C, PB, G], mybir.dt.float32, isOutput=False)
    o = nc.declare_dram_parameter("o", [C, V, PB, HOUT], mybir.dt.float32,
                                  isOutput=True)
    with ExitStack() as ctx:
        block = ctx.enter_context(nc.Block(no_gpsimd_drain=True))
        ld_sems = [ctx.enter_context(nc.semaphore(f"ld{c}")) for c in range(C)]
        full_sems = [ctx.enter_context(nc.semaphore(f"full{c}")) for c in range(C)]
        st_sem = ctx.enter_context(nc.semaphore("st"))
        t = ctx.enter_context(nc.sbuf_tensor("t", [PB, C * EXT], mybir.dt.float32))

        def slot(c, k):
            return t[:, c * EXT + k * G:c * EXT + (k + 1) * G]

        def store(eng, c):
            eng.wait_ge(full_sems[c], 3)
            base = c * EXT
            # A: all partitions, cols [0:WA)
            dst = o[c][:, :, 0:WA].rearrange("v p col -> p v col")
            src = t[:, base:base + WA].unsqueeze(1).broadcast_to((PB, V, WA))
            eng.dma_start(dst, src).then_inc(st_sem, 16)
            # B: partitions [0:120), cols [WA:4096)
            dst = o[c][:, 0:120, WA:HOUT].rearrange("v p col -> p v col")
            src = t[0:120, base + WA:base + EXT]
            src = src.unsqueeze(1).broadcast_to((120, V, EXT - WA))
            eng.dma_start(dst, src).then_inc(st_sem, 16)
            # C: partitions [120:128), cols [WA:4096)
            dst = o[c][:, 120:PB, WA:HOUT].rearrange("v p col -> p v col")
            src = t[120:PB, base + WA:base + EXT]
            src = src.unsqueeze(1).broadcast_to((PB - 120, V, EXT - WA))
            eng.dma_start(dst, src).then_inc(st_sem, 16)

        @block.vector
        def _(vector: bass.BassEngine):
            for c in range(C):
                vector.wait_ge(ld_sems[c], 16)
                vector.tensor_copy(slot(c, 1), slot(c, 0)).then_inc(
                    full_sems[c], 1)
                vector.tensor_copy(slot(c, 3), slot(c, 0)).then_inc(
                    full_sems[c], 1)

        @block.scalar
        def _(scalar: bass.BassEngine):
            for c in range(C):
                scalar.dma_start(slot(c, 0), g[c]).then_inc(ld_sems[c], 16)
            for c in range(C):
                scalar.wait_ge(ld_sems[c], 16)
                scalar.copy(slot(c, 2), slot(c, 0)).then_inc(full_sems[c], 1)
                if c in (1, 3):
                    store(scalar, c)

        @block.sync
        def _(sync: bass.BassEngine):
            store(sync, 0)
            store(sync, 2)
            sync.wait_ge(st_sem, 16 * 3 * C)
    return nc


def _build_v6() -> bass.Bass:
    """v5 with the replication split DVE(2 slots) + scalar(1 slot); the
    gpsimd Q7 copy (4.1us for 512KB) was gating the first store."""
    EXT = HOUT
    nc = bass.Bass()
    g = nc.declare_dram_parameter("g", [C, PB, G], mybir.dt.float32, isOutput=False)
    o = nc.declare_dram_parameter("o", [C, V, PB, HOUT], mybir.dt.float32,
                                  isOutput=True)
    with ExitStack() as ctx:
        block = ctx.enter_context(nc.Block(no_gpsimd_drain=True))
        ld_sems = [ctx.enter_context(nc.semaphore(f"ld{c}")) for c in range(C)]
        full_sems = [ctx.enter_context(nc.semaphore(f"full{c}")) for c in range(C)]
        st_sem = ctx.enter_context(nc.semaphore("st"))
        t = ctx.enter_context(nc.sbuf_tensor("t", [PB, C * EXT], mybir.dt.float32))

        def slot(c, k):
            return t[:, c * EXT + k * G:c * EXT + (k + 1) * G]

        def store(eng, c):
            eng.wait_ge(full_sems[c], 3)
            dst = o[c].rearrange("v p col -> p v col")
            src = t[:, c * EXT:(c + 1) * EXT]
            src = src.unsqueeze(1).broadcast_to((PB, V, EXT))
            eng.dma_start(dst, src).then_inc(st_sem, 16)

        @block.vector
        def _(vector: bass.BassEngine):
            for c in range(C):
                vector.wait_ge(ld_sems[c], 16)
                vector.tensor_copy(slot(c, 1), slot(c, 0)).then_inc(
                    full_sems[c], 1)
                vector.tensor_copy(slot(c, 3), slot(c, 0)).then_inc(
                    full_sems[c], 1)

        @block.scalar
        def _(scalar: bass.BassEngine):
            for c in range(C):
                scalar.dma_start(slot(c, 0), g[c]).then_inc(ld_sems[c], 16)
            for c in range(C):
                scalar.wait_ge(ld_sems[c], 16)
                scalar.copy(slot(c, 2), slot(c, 0)).then_inc(full_sems[c], 1)
                if c in (1, 3):
                    store(scalar, c)

        @block.sync
        def _(sync: bass.BassEngine):
            store(sync, 0)
            store(sync, 2)
            sync.wait_ge(st_sem, 16 * C)
    return nc


def _build_v5() -> bass.Bass:
    """v4 + (a) stores split across both HWDGE rings (sync: c0,c2;
    scalar: c1,c3) so descriptor fetch/decode overlaps across rings,
    and (b) the 3 replication copies per channel run on DVE, scalar and
    gpsimd in parallel, shortening the ld0 -> replicate -> store chain.
    """
    EXT = HOUT
    nc = bass.Bass()
    g = nc.declare_dram_parameter("g", [C, PB, G], mybir.dt.float32, isOutput=False)
    o = nc.declare_dram_parameter("o", [C, V, PB, HOUT], mybir.dt.float32,
                                  isOutput=True)
    with ExitStack() as ctx:
        block = ctx.enter_context(nc.Block())
        ld_sems = [ctx.enter_context(nc.semaphore(f"ld{c}")) for c in range(C)]
        full_sems = [ctx.enter_context(nc.semaphore(f"full{c}")) for c in range(C)]
        st_sem = ctx.enter_context(nc.semaphore("st"))
        t = ctx.enter_context(nc.sbuf_tensor("t", [PB, C * EXT], mybir.dt.float32))

        def slot(c, k):
            return t[:, c * EXT + k * G:c * EXT + (k + 1) * G]

        def store(eng, c):
            eng.wait_ge(full_sems[c], 3)
            dst = o[c].rearrange("v p col -> p v col")
            src = t[:, c * EXT:(c + 1) * EXT]
            src = src.unsqueeze(1).broadcast_to((PB, V, EXT))
            eng.dma_start(dst, src).then_inc(st_sem, 16)

        @block.vector
        def _(vector: bass.BassEngine):
            for c in range(C):
                vector.wait_ge(ld_sems[c], 16)
                vector.tensor_copy(slot(c, 1), slot(c, 0)).then_inc(
                    full_sems[c], 1)

        @block.gpsimd
        def _(gpsimd: bass.BassEngine):
            for c in range(C):
                gpsimd.wait_ge(ld_sems[c], 16)
                gpsimd.tensor_copy(slot(c, 3), slot(c, 0)).then_inc(
                    full_sems[c], 1)

        @block.scalar
        def _(scalar: bass.BassEngine):
            for c in range(C):
                scalar.dma_start(slot(c, 0), g[c]).then_inc(ld_sems[c], 16)
            for c in range(C):
                scalar.wait_ge(ld_sems[c], 16)
                scalar.copy(slot(c, 2), slot(c, 0)).then_inc(full_sems[c], 1)
                if c in (1, 3):
                    store(scalar, c)

        @block.sync
        def _(sync: bass.BassEngine):
            store(sync, 0)
            store(sync, 2)
            sync.wait_ge(st_sem, 16 * C)
    return nc


def _build_v4() -> bass.Bass:
    """Host pre-rolls columns, so SBUF rows are already output-ordered.

    Per core: load the (C, PB, G) band (2MB) via the scalar HWDGE ring,
    DVE-replicates each channel row 4x side-by-side (4KB -> 16KB), then
    one store DMA per channel writes the (V, PB, 16KB) block with the
    v-replication done by a stride-0 source dim. Descriptors are 16KB
    contiguous -> minimal per-packet overhead; the HBM write cap
    (~358 GB/s/core) becomes the binding limit.
    """
    EXT = HOUT  # 4096 replicated columns per channel
    nc = bass.Bass()
    g = nc.declare_dram_parameter("g", [C, PB, G], mybir.dt.float32, isOutput=False)
    o = nc.declare_dram_parameter("o", [C, V, PB, HOUT], mybir.dt.float32,
                                  isOutput=True)
    with ExitStack() as ctx:
        block = ctx.enter_context(nc.Block())
        ld_sems = [ctx.enter_context(nc.semaphore(f"ld{c}")) for c in range(C)]
        dve_sem = ctx.enter_context(nc.semaphore("dve"))
        st_sem = ctx.enter_context(nc.semaphore("st"))
        t = ctx.enter_context(nc.sbuf_tensor("t", [PB, C * EXT], mybir.dt.float32))

        @block.scalar
        def _(scalar: bass.BassEngine):
            # Loads ride the Act HWDGE ring; stores ride the SP ring, so
            # the 4 loads never queue behind 8MB store descriptors.
            for c in range(C):
                scalar.dma_start(t[:, c * EXT:c * EXT + G], g[c]).then_inc(
                    ld_sems[c], 16)

        @block.vector
        def _(vector: bass.BassEngine):
            for c in range(C):
                base = c * EXT
                vector.wait_ge(ld_sems[c], 16)
                inst = None
                for k in range(1, EXT // G):
                    inst = vector.tensor_copy(
                        t[:, base + k * G:base + (k + 1) * G],
                        t[:, base:base + G],
                    )
                inst.then_inc(dve_sem, 1)

        @block.sync
        def _(sync: bass.BassEngine):
            for c in range(C):
                sync.wait_ge(dve_sem, c + 1)
                # dst (p, v, col): one 8MB DMA, 16KB contiguous per (p, v).
                dst = o[c].rearrange("v p col -> p v col")
                src = t[:, c * EXT:(c + 1) * EXT]
                src = src.unsqueeze(1).broadcast_to((PB, V, EXT))
                sync.dma_start(dst, src).then_inc(st_sem, 16)
            sync.wait_ge(st_sem, 16 * C)
    return nc


def _build_v3(xs: int) -> bass.Bass:
    """One SPMD program, specialized on the column shift xs.

    Raw bass (not Tile): the static-DMA lowering in this toolchain only
    supports a single sync-wait per DMA instruction, so sequencer-side
    wait_ge + per-channel load semaphores are used instead of Tile's
    auto-generated multi-sem waits.

    v3: each channel's grid row is replicated 5x side-by-side in SBUF
    (DVE copies — otherwise idle), so every output row is one contiguous
    16KB descriptor ext[p, xs:xs+4096] and each (c, v) block is a single
    2MB store DMA with maximal descriptor size. (v1 used 2KB segmented
    descriptors from the column roll; engines ran at 23.3/27 GB/s and
    the slow 16th engine set a 117us makespan.)
    """
    EXT = G + HOUT  # 5120 replicated columns per channel
    nc = bass.Bass()
    g = nc.declare_dram_parameter("g", [C, PB, G], mybir.dt.float32, isOutput=False)
    o = nc.declare_dram_parameter("o", [C, V, PB, HOUT], mybir.dt.float32, isOutput=True)
    with ExitStack() as ctx:
        block = ctx.enter_context(nc.Block())
        ld_sems = [ctx.enter_context(nc.semaphore(f"ld{c}")) for c in range(C)]
        dve_sem = ctx.enter_context(nc.semaphore("dve"))
        st_sem = ctx.enter_context(nc.semaphore("st"))
        t = ctx.enter_context(nc.sbuf_tensor("t", [PB, C * EXT], mybir.dt.float32))

        @block.vector
        def _(vector: bass.BassEngine):
            for c in range(C):
                base = c * EXT
                vector.wait_ge(ld_sems[c], 16)
                for k in range(1, EXT // G):
                    inst = vector.tensor_copy(
                        t[:, base + k * G:base + (k + 1) * G],
                        t[:, base:base + G],
                    )
                inst.then_inc(dve_sem, 1)

        @block.sync
        def _(sync: bass.BassEngine):
            for c in range(C):
                sync.dma_start(t[:, c * EXT:c * EXT + G], g[c]).then_inc(
                    ld_sems[c], 16)
            for c in range(C):
                sync.wait_ge(dve_sem, c + 1)
                for v in range(V):
                    src = t[:, c * EXT + xs:c * EXT + xs + HOUT]
                    sync.dma_start(o[c, v], src).then_inc(st_sem, 16)
            sync.wait_ge(st_sem, 16 * C * V)
    return nc


def _build_v1(xs: int) -> bass.Bass:
    """v1 (kept for reference): column roll via 2-segment stores."""
    nc = bass.Bass()
    g = nc.declare_dram_parameter("g", [C, PB, G], mybir.dt.float32, isOutput=False)
    o = nc.declare_dram_parameter("o", [C, V, PB, HOUT], mybir.dt.float32, isOutput=True)
    L = G - xs
    with ExitStack() as ctx:
        block = ctx.enter_context(nc.Block())
        ld_sems = [ctx.enter_context(nc.semaphore(f"ld{c}")) for c in range(C)]
        st_sem = ctx.enter_context(nc.semaphore("st"))
        t = ctx.enter_context(nc.sbuf_tensor("t", [PB, C * G], mybir.dt.float32))

        @block.sync
        def _(sync: bass.BassEngine):
            for c in range(C):
                sync.dma_start(t[:, c * G:(c + 1) * G], g[c]).then_inc(
                    ld_sems[c], 16)
            nstores = 0
            for c in range(C):
                sync.wait_ge(ld_sems[c], 16)
                for v in range(V):
                    # out[c, v, p, r*1024 + b] = t[p, c*1024 + (xs + b) % 1024]
                    dst = o[c, v].rearrange("p (r col) -> p r col", col=G)
                    srcA = t[:, c * G + xs:(c + 1) * G]
                    srcA = srcA.unsqueeze(1).broadcast_to((PB, R, L))
                    sync.dma_start(dst[:, :, 0:L], srcA).then_inc(st_sem, 16)
                    nstores += 1
                    if xs:
                        srcB = t[:, c * G:c * G + xs]
                        srcB = srcB.unsqueeze(1).broadcast_to((PB, R, xs))
                        sync.dma_start(dst[:, :, L:G], srcB).then_inc(st_sem, 16)
                        nstores += 1
            sync.wait_ge(st_sem, 16 * nstores)
    return nc


def _get_nc(xs: int) -> bass.Bass:
    if KERNEL_VERSION >= 4:
        xs = 0  # v4+ rolls columns on the host; the NEFF is xs-independent
    key = (KERNEL_VERSION, xs)
    if key not in _NC_CACHE:
        _NC_CACHE[key] = _build(xs)
    return _NC_CACHE[key]


def kernel(grid, coordinate_start, h, w, support_resolution_h, support_resolution_w,
           **_unused):
    grid = np.asarray(grid, dtype=np.float32)
    cs = np.asarray(coordinate_start).astype(np.int64)
    xs = int(cs[0]) % G
    ys = int(cs[1]) % G
    assert grid.shape == (1, C, G, G), grid.shape
    assert int(h) == HOUT and int(w) == HOUT
    assert int(support_resolution_h) == G and int(support_resolution_w) == G

    g0 = grid[0]  # (C, G, G)
    if KERNEL_VERSION >= 4:
        # v4 does no on-device column roll: pre-roll the whole grid once
        # so band[c, p, j] = g0[c, rows[p], (xs + j) % G].
        g0 = np.ascontiguousarray(np.roll(g0, -xs, axis=2))
    in_maps = []
    for k in range(NCORES):
        rows = (ys + PB * k + np.arange(PB)) % G
        band = np.ascontiguousarray(g0[:, rows, :])  # (C, PB, G)
        in_maps.append({"g": band})

    nc = _get_nc(xs)
    res = run_bass_kernel_spmd(nc, in_maps, core_ids=list(range(NCORES)),
                               trace=TRACE)
    global LAST_RESULT
    LAST_RESULT = res

    full = np.empty((1, C, HOUT, HOUT), dtype=np.float32)
    for k in range(NCORES):
        r = np.asarray(res.results[k]["o"])  # (C, V, PB, HOUT)
        for v in range(V):
            base = v * G + PB * k
            full[0, :, base:base + PB, :] = r[:, v]
    return full

